# revision 7
# baseline (speedup 1.0000x reference)
"""Trainium2 Bass kernel for nn_BasicNCAModel (neural cellular automaton).

Sharding: data-parallel over batch B=8 across 8 NeuronCores (1 image/core).
kernel() takes full inputs, shards per image on the host, runs the SPMD Bass
module via run_bass_kernel_spmd (PJRT under axon), and reassembles.

Per-core design v2 (hardcoded for B=8, H=W=128, C=24, hidden=128, steps=8):
  - x lives ONLY as an fp16 master (x16, channel-major with halo: partition
    32g+c holds channel c of image rows [32g-1, 32g+32], 34 rows x 132 pitch).
    Updated in place; an fp8 shadow (x8) feeds the stack builds.
  - mm1 runs in fp8e4m3 (max 240): per-tap matrices A_t are scaled by 4,
    the fire coefficient is 224 = 4*56, bias is 4*(b1-56), and W2^T is
    scaled by 1/4, so dx is exact up to quantization. Masked pixels get
    relu(4(h+b1) - 224) = 0 (fire trick).
  - Per group a fp8 dx-stack (ping-ponged per step) holds rows 24d+c =
    x8 shifted by dx=d-1 (baked via contiguous DMA), row 72 = fire (DMA'd
    straight from DRAM per step), rows 73..127 = 0 (K padded to 128 for
    FWL full-rate matmuls). mm1 = 3 dy-matmuls per 512-pixel tile (dy via
    +-PITCH in the rhs AP); mm2 (fp16) is column-tiled so group g's dx
    lands at PSUM partitions 32g..32g+31 and x += dx is one in-place DVE
    add per tile.
  - Engine balance: Scalar = relu evac cols 0:256 + wrap-column fills +
    fp16->fp8 cast; Vector = STT evac cols 256:512 + the x update; GpSimd =
    stack-halo edge copies + fire DMAs + half the stack-slice DMAs; Sync =
    the other half + streamed output DMAs during the last step.
"""

import sys

if "/opt/trn_rl_repo" not in sys.path:
    sys.path.insert(0, "/opt/trn_rl_repo")

import ml_dtypes
import numpy as np

C = 24
NIC = 4
H = 128
WID = 128
HID = 128
STEPS = 8
B = 8
G = 4          # row groups
RG = 32        # image rows per group
PITCH = 132    # free-dim row pitch (130 used + 2 pad)
GROWS = 34     # rows incl halo
FB = GROWS * PITCH
TW = 512       # pixel tile = 4 image rows * 128 cols
JT = RG // 4   # tiles per group per step
WSCALE = 4.0   # weight scale so fire coeff fits fp8e4m3 (max 240)
M_FIRE = 56.0  # fire margin; coeff = WSCALE*M_FIRE = 224 (exact in fp8)
ASPL = 256     # evac split: scalar does [0:ASPL], vector [ASPL:512]

F8NP = ml_dtypes.float8_e4m3

_CACHE = {}


def _build_module():
    from concourse import bacc, mybir, tile

    f32 = mybir.dt.float32
    f16 = mybir.dt.float16
    f8 = mybir.dt.float8e4
    Alu = mybir.AluOpType
    Act = mybir.ActivationFunctionType

    nc = bacc.Bacc(
        "TRN2",
        target_bir_lowering=False,
        debug=False,
        enable_asserts=False,
        num_devices=8,
    )

    x16in = nc.dram_tensor("x16in", [128, FB], f16, kind="ExternalInput").ap()
    stk0in = nc.dram_tensor("stk0in", [128, G * FB], f8, kind="ExternalInput").ap()
    firein = nc.dram_tensor("firein", [128, 4096], f8, kind="ExternalInput").ap()
    apA = nc.dram_tensor("apA", [128, 384], f8, kind="ExternalInput").ap()
    w2p = nc.dram_tensor("w2p", [128, 32], f16, kind="ExternalInput").ap()
    b1col = nc.dram_tensor("b1col", [128, 1], f32, kind="ExternalInput").ap()
    xout = nc.dram_tensor("xout", [128, 4096], f16, kind="ExternalOutput").ap()

    with tile.TileContext(nc) as tc:
        import contextlib

        with contextlib.ExitStack() as ctx:
            sing = ctx.enter_context(tc.tile_pool(name="sing", bufs=1))
            hpool = ctx.enter_context(tc.tile_pool(name="h", bufs=6, space="PSUM"))
            dxpool = ctx.enter_context(tc.tile_pool(name="dx", bufs=2, space="PSUM"))
            hsb = ctx.enter_context(tc.tile_pool(name="hsb", bufs=8))

            x16 = sing.tile([128, FB], f16)
            x8 = sing.tile([128, FB], f8)
            A = sing.tile([128, 384], f8)
            W2s = sing.tile([128, 32], f16)
            zeros = sing.tile([128, TW - ASPL], f32)
            b1c = sing.tile([128, 1], f32)
            stk = [
                [sing.tile([128, FB], f8, name=f"stk_{g}_{p}") for p in range(2)]
                for g in range(G)
            ]

            # ---- preload: stack rows 0..19 (first tiles) race ahead on all
            # three DMA queues so their fixed costs overlap; x16 (needed at
            # the first update) and stack rows 20..33 queue behind them ----
            nc.scalar.dma_start(A[:], apA[:])
            nc.scalar.dma_start(W2s[:], w2p[:])
            nc.scalar.dma_start(b1c[:], b1col[:])
            PSPLIT = 2640  # rows 0..19 first (covers the first ~4 tiles)
            _eng = [nc.sync, nc.gpsimd, nc.scalar, nc.sync]
            for g in range(G):
                _eng[g].dma_start(
                    stk[g][0][:, 0:PSPLIT], stk0in[:, g * FB : g * FB + PSPLIT]
                )
            nc.gpsimd.dma_start(x16[:], x16in[:])
            for g in range(G):
                _eng[(g + 1) % 3].dma_start(
                    stk[g][0][:, PSPLIT:FB],
                    stk0in[:, g * FB + PSPLIT : (g + 1) * FB],
                )
            for g in range(G):
                # odd-step stacks: zero rows 64+ (fire + K-padding; rows
                # 64..72 get overwritten by slice/fire DMAs each step)
                nc.vector.memset(stk[g][1][64:128, :], 0.0)
            nc.vector.memset(zeros[:], 0.0)

            x16v = x16[:].rearrange("p (r w) -> p r w", w=PITCH)
            x8v = x8[:].rearrange("p (r w) -> p r w", w=PITCH)
            xo3 = xout.rearrange("p (r w) -> p r w", w=128)
            # stack slice boundaries (flat y): rows 1-4 | 5-16 | 17-24 | 25-32
            SLICES = [(132, 660), (660, 2244), (2244, 3300), (3300, 4356)]
            # stack slice sl becomes buildable once these casts are done;
            # JORD puts j=5,4 early so slice 2 (needed by next-step j=3)
            # fires 4 tiles before the step ends
            TRIGGER = {0: 0, 1: 1, 4: 2, 7: 3}  # cast j -> slice index
            JORD = [2, 3, 1, 5, 4, 0, 6, 7]

            def emit_slice(s, sl):
                """Stack-build DMAs (x8 -> stacks, dx baked) for step s+1."""
                nb = (s + 1) % 2
                ylo, yhi = SLICES[sl]
                for g in range(G):
                    sg = stk[g][nb]
                    for d in range(3):
                        eng = [nc.sync, nc.gpsimd, nc.scalar][(g + d) % 3]
                        eng.dma_start(
                            sg[24 * d : 24 * d + 24, ylo:yhi],
                            x8[32 * g : 32 * g + 24, ylo + d : yhi + d],
                        )

            def emit_fire(s):
                """Fire rows for step s+1, straight from DRAM."""
                nb = (s + 1) % 2
                for g in range(G):
                    s3 = stk[g][nb][:].rearrange("p (r w) -> p r w", w=PITCH)
                    nc.gpsimd.dma_start(
                        s3[72:73, 1:33, 0:128],
                        firein[32 * g + s + 1 : 32 * g + s + 2, :],
                    )

            def emit_edges(s):
                """Neighbor-stack halo rows for step s+1 (same partitions)."""
                nb = (s + 1) % 2
                for g in range(G):
                    sg = stk[g][nb]
                    sm = stk[(g - 1) % G][nb]
                    sp = stk[(g + 1) % G][nb]
                    nc.gpsimd.tensor_copy(sg[:73, 0:132], sm[:73, 4224:4356])
                    nc.gpsimd.tensor_copy(sg[:73, 4356:4488], sp[:73, 132:264])

            for s in range(STEPS):
                sb, nb = s % 2, (s + 1) % 2
                if s + 1 < STEPS:
                    emit_fire(s)
                stks = [
                    stk[g][sb][:].rearrange("p (r w) -> p r w", w=PITCH)
                    for g in range(G)
                ]

                def mm2_update(j, hss):
                    r0 = 4 * j + 1
                    dxt = dxpool.tile(
                        [128, TW], f32, tag="dx", name=f"dx_{s}_{j}"
                    )
                    for g in range(G):
                        nc.tensor.matmul(
                            dxt[32 * g : 32 * g + 32, :],
                            W2s[:],
                            hss[g][:],
                            start=True,
                            stop=True,
                            tile_position=(0, 32 * g),
                        )
                    # x += dx, in place (dx rows 24..31 of each band are 0)
                    dx3 = dxt[:].rearrange("p (a b) -> p a b", b=128)
                    nc.vector.tensor_tensor(
                        x16v[:, r0 : r0 + 4, 1:129],
                        dx3,
                        x16v[:, r0 : r0 + 4, 1:129],
                        Alu.add,
                    )
                    if s + 1 < STEPS:
                        # wrap columns then fp8 shadow of the updated rows
                        nc.scalar.activation(
                            x16v[:, r0 : r0 + 4, 0:1],
                            x16v[:, r0 : r0 + 4, 128:129],
                            Act.Copy,
                        )
                        nc.scalar.activation(
                            x16v[:, r0 : r0 + 4, 129:130],
                            x16v[:, r0 : r0 + 4, 1:2],
                            Act.Copy,
                        )
                        nc.scalar.activation(
                            x8v[:, r0 : r0 + 4, 0:130],
                            x16v[:, r0 : r0 + 4, 0:130],
                            Act.Copy,
                        )
                        if j in TRIGGER:
                            emit_slice(s, TRIGGER[j])
                    else:
                        nc.sync.dma_start(
                            xo3[:, 4 * j : 4 * j + 4, :],
                            x16v[:, r0 : r0 + 4, 1:129],
                        )

                prev = None
                for j in JORD:
                    r0 = 4 * j + 1
                    hts = [
                        hpool.tile([128, TW], f32, tag="h", name=f"h_{s}_{j}_{g}")
                        for g in range(G)
                    ]
                    # mm1: 3 dy-matmuls per group (fp8, K padded to 128);
                    # dy shift via the rhs AP, dx via the stack rows
                    for di, dy in enumerate((-1, 0, 1)):
                        lhsT = A[0:128, 128 * (dy + 1) : 128 * (dy + 2)]
                        for g in range(G):
                            rhs = stks[g][0:128, r0 + dy : r0 + dy + 4, 0:128]
                            nc.tensor.matmul(
                                hts[g][:, :],
                                lhsT,
                                rhs,
                                start=(di == 0),
                                stop=(di == 2),
                                tile_position=(0, 0),
                            )
                    hss = []
                    for g in range(G):
                        ht = hts[g]
                        hs = hsb.tile(
                            [128, TW], f16, tag="hsb", name=f"hs_{s}_{j}_{g}"
                        )
                        # h' = relu(h + 4(b1-56)); dy=0 fire row added 224*fire
                        nc.scalar.activation(
                            hs[:, :ASPL],
                            ht[:, :ASPL],
                            Act.Relu,
                            bias=b1c[:],
                        )
                        nc.vector.scalar_tensor_tensor(
                            hs[:, ASPL:],
                            ht[:, ASPL:],
                            b1c[:],
                            zeros[:],
                            Alu.add,
                            Alu.max,
                        )
                        hss.append(hs)
                    if prev is not None:
                        mm2_update(*prev)
                    prev = (j, hss)
                mm2_update(*prev)
                if s + 1 < STEPS:
                    emit_edges(s)

    nc.compile()
    return nc


def _get_module():
    if "nc" not in _CACHE:
        _CACHE["nc"] = _build_module()
    return _CACHE["nc"]


def _prep_weights(w1, w2, W1, b1, W2):
    A = np.zeros((9, HID, C), np.float32)
    for t in range(9):
        dy, dxx = t // 3 - 1, t % 3 - 1
        A[t] = (
            W1[:, 24:48] * w1[dy + 1, dxx + 1, 0][None, :]
            + W1[:, 48:72] * w2[dy + 1, dxx + 1, 0][None, :]
        )
    A[4] += W1[:, :24]
    apack = np.zeros((128, 384), np.float32)
    for d in range(3):
        for dyi in range(3):
            t = dyi * 3 + d
            apack[24 * d : 24 * d + 24, 128 * dyi : 128 * dyi + 128] = (
                WSCALE * A[t].T
            )
    apack[72, 128:256] = WSCALE * M_FIRE  # 224, exact in fp8e4m3
    w2pk = np.zeros((128, 32), np.float32)
    w2pk[:, NIC:C] = W2[NIC:C].T / WSCALE
    b1c = (WSCALE * (b1 - M_FIRE)).reshape(128, 1).astype(np.float32)
    return apack.astype(F8NP), w2pk.astype(np.float16), b1c


def _pack_x(ximg):
    """[128,128,24] image -> [128, FB] haloed channel-major."""
    xin = np.zeros((128, FB), np.float32)
    cols = (np.arange(-1, 129)) % WID
    for g in range(G):
        rows = (np.arange(-1, 33) + 32 * g) % H
        blk = ximg[rows][:, cols, :]  # [34, 130, 24]
        buf = np.zeros((24, GROWS, PITCH), np.float32)
        buf[:, :, :130] = np.transpose(blk, (2, 0, 1))
        xin[32 * g : 32 * g + 24] = buf.reshape(24, FB)
    return xin


def _unpack_x(xo):
    """[128, 4096] -> [128,128,24] image."""
    img = np.empty((H, WID, C), np.float32)
    for g in range(G):
        blk = xo[32 * g : 32 * g + 24].astype(np.float32).reshape(24, RG, WID)
        img[32 * g : 32 * g + 32] = np.transpose(blk, (1, 2, 0))
    return img


def _build_stack0(x8_0, fire0):
    """Host: step-0 stacks [128, G*FB] fp8 (x8_0 halos make edges free)."""
    stkin = np.zeros((128, G * FB), F8NP)
    for g in range(G):
        blk = stkin[:, g * FB : (g + 1) * FB]
        for d in range(3):
            blk[24 * d : 24 * d + 24, : FB - d] = x8_0[
                32 * g : 32 * g + 24, d:FB
            ]
        fr = fire0[32 * g : 32 * g + 32].reshape(RG * WID)
        f2 = blk[72].reshape(GROWS, PITCH)
        f2[1:33, 0:128] = fr.reshape(RG, WID)
    return stkin


def _make_in_maps(x, w1, w2, W1, b1, W2, rand_u):
    apack, w2pk, b1c = _prep_weights(w1, w2, W1, b1, W2)
    in_maps = []
    for b in range(B):
        fire = (rand_u[:, b, :, :, 0] < 0.5).astype(F8NP)  # [steps, H, W]
        firein = np.zeros((128, 4096), F8NP)
        for g in range(G):
            for s in range(STEPS):
                firein[32 * g + s] = fire[s, 32 * g : 32 * g + 32].reshape(4096)
        x16 = _pack_x(np.asarray(x[b], np.float32)).astype(np.float16)
        x8_0 = x16.astype(F8NP)
        in_maps.append(
            {
                "x16in": x16,
                "stk0in": _build_stack0(x8_0, fire[0]),
                "firein": firein,
                "apA": apack,
                "w2p": w2pk,
                "b1col": b1c,
            }
        )
    return in_maps


def kernel(x, w1, w2, W1, b1, W2, rand_u, steps, **kw):
    from concourse.bass_utils import run_bass_kernel_spmd

    assert int(steps) == STEPS
    x = np.asarray(x, np.float32)
    in_maps = _make_in_maps(
        x,
        np.asarray(w1, np.float32),
        np.asarray(w2, np.float32),
        np.asarray(W1, np.float32),
        np.asarray(b1, np.float32),
        np.asarray(W2, np.float32),
        np.asarray(rand_u, np.float32),
    )
    nc = _get_module()
    res = run_bass_kernel_spmd(nc, in_maps, core_ids=list(range(B)))
    _CACHE["last_results"] = res
    out = np.empty((B, H, WID, C), np.float32)
    for b in range(B):
        out[b] = _unpack_x(res.results[b]["xout"])
    return out


# revision 8
# speedup vs baseline: 1.0815x; 1.0815x over previous
"""Trainium2 Bass kernel for nn_BasicNCAModel (neural cellular automaton).

Sharding: data-parallel over batch B=8 across 8 NeuronCores (1 image/core).
kernel() takes full inputs, shards per image on the host, runs the SPMD Bass
module via run_bass_kernel_spmd (PJRT under axon), and reassembles.

Per-core design v2 (hardcoded for B=8, H=W=128, C=24, hidden=128, steps=8):
  - x lives ONLY as an fp16 master (x16, channel-major with halo: partition
    32g+c holds channel c of image rows [32g-1, 32g+32], 34 rows x 132 pitch).
    Updated in place; an fp8 shadow (x8) feeds the stack builds.
  - mm1 runs in fp8e4m3 (max 240): per-tap matrices A_t are scaled by 4,
    the fire coefficient is 224 = 4*56, bias is 4*(b1-56), and W2^T is
    scaled by 1/4, so dx is exact up to quantization. Masked pixels get
    relu(4(h+b1) - 224) = 0 (fire trick).
  - Per group a fp8 dx-stack (ping-ponged per step) holds rows 24d+c =
    x8 shifted by dx=d-1 (baked via contiguous DMA), row 72 = fire (DMA'd
    straight from DRAM per step), rows 73..127 = 0 (K padded to 128 for
    FWL full-rate matmuls). mm1 = 3 dy-matmuls per 512-pixel tile (dy via
    +-PITCH in the rhs AP); mm2 (fp16) is column-tiled so group g's dx
    lands at PSUM partitions 32g..32g+31 and x += dx is one in-place DVE
    add per tile.
  - Engine balance: Scalar = relu evac cols 0:256 + wrap-column fills +
    fp16->fp8 cast; Vector = STT evac cols 256:512 + the x update; GpSimd =
    stack-halo edge copies + fire DMAs + half the stack-slice DMAs; Sync =
    the other half + streamed output DMAs during the last step.
"""

import sys

if "/opt/trn_rl_repo" not in sys.path:
    sys.path.insert(0, "/opt/trn_rl_repo")

import ml_dtypes
import numpy as np

C = 24
NIC = 4
H = 128
WID = 128
HID = 128
STEPS = 8
B = 8
G = 4          # row groups
RG = 32        # image rows per group
PITCH = 132    # free-dim row pitch (130 used + 2 pad)
GROWS = 34     # rows incl halo
FB = GROWS * PITCH
TW = 512       # pixel tile = 4 image rows * 128 cols
JT = RG // 4   # tiles per group per step
WSCALE = 4.0   # weight scale so fire coeff fits fp8e4m3 (max 240)
M_FIRE = 56.0  # fire margin; coeff = WSCALE*M_FIRE = 224 (exact in fp8)
ASPL = 256     # evac split: scalar does [0:ASPL], vector [ASPL:512]

F8NP = ml_dtypes.float8_e4m3

_CACHE = {}


def _build_module():
    from concourse import bacc, mybir, tile

    f32 = mybir.dt.float32
    f16 = mybir.dt.float16
    f8 = mybir.dt.float8e4
    Alu = mybir.AluOpType
    Act = mybir.ActivationFunctionType

    nc = bacc.Bacc(
        "TRN2",
        target_bir_lowering=False,
        debug=False,
        enable_asserts=False,
        num_devices=8,
    )

    x16in = nc.dram_tensor("x16in", [128, FB], f16, kind="ExternalInput").ap()
    stk0in = nc.dram_tensor("stk0in", [128, G * FB], f8, kind="ExternalInput").ap()
    firein = nc.dram_tensor("firein", [128, 4096], f8, kind="ExternalInput").ap()
    apA = nc.dram_tensor("apA", [128, 384], f8, kind="ExternalInput").ap()
    w2p = nc.dram_tensor("w2p", [128, 32], f16, kind="ExternalInput").ap()
    b1col = nc.dram_tensor("b1col", [128, 1], f32, kind="ExternalInput").ap()
    xout = nc.dram_tensor("xout", [128, 4096], f16, kind="ExternalOutput").ap()

    with tile.TileContext(nc) as tc:
        import contextlib

        with contextlib.ExitStack() as ctx:
            sing = ctx.enter_context(tc.tile_pool(name="sing", bufs=1))
            hpool = ctx.enter_context(tc.tile_pool(name="h", bufs=6, space="PSUM"))
            dxpool = ctx.enter_context(tc.tile_pool(name="dx", bufs=2, space="PSUM"))
            hsb = ctx.enter_context(tc.tile_pool(name="hsb", bufs=8))

            x16 = sing.tile([128, FB], f16)
            x8 = sing.tile([128, FB], f8)
            A = sing.tile([128, 384], f8)
            W2s = sing.tile([128, 32], f16)
            zeros = sing.tile([128, TW - ASPL], f32)
            b1c = sing.tile([128, 1], f32)
            stk = [
                [sing.tile([128, FB], f8, name=f"stk_{g}_{p}") for p in range(2)]
                for g in range(G)
            ]

            # ---- preload: stack rows 0..19 (first tiles) race ahead on all
            # three DMA queues so their fixed costs overlap; x16 (needed at
            # the first update) and stack rows 20..33 queue behind them ----
            nc.scalar.dma_start(A[:], apA[:])
            nc.scalar.dma_start(W2s[:], w2p[:])
            nc.scalar.dma_start(b1c[:], b1col[:])
            PSPLIT = 2640  # rows 0..19 first (covers the first ~4 tiles)
            _eng = [nc.sync, nc.gpsimd, nc.scalar, nc.sync]
            for g in range(G):
                _eng[g].dma_start(
                    stk[g][0][:, 0:PSPLIT], stk0in[:, g * FB : g * FB + PSPLIT]
                )
            nc.gpsimd.dma_start(x16[:], x16in[:])
            for g in range(G):
                _eng[(g + 1) % 3].dma_start(
                    stk[g][0][:, PSPLIT:FB],
                    stk0in[:, g * FB + PSPLIT : (g + 1) * FB],
                )
            for g in range(G):
                # odd-step stacks: zero rows 64+ (fire + K-padding; rows
                # 64..72 get overwritten by slice/fire DMAs each step)
                nc.vector.memset(stk[g][1][64:128, :], 0.0)
            nc.vector.memset(zeros[:], 0.0)

            x16v = x16[:].rearrange("p (r w) -> p r w", w=PITCH)
            x8v = x8[:].rearrange("p (r w) -> p r w", w=PITCH)
            xo3 = xout.rearrange("p (r w) -> p r w", w=128)
            # stack slice boundaries (flat y): rows 1-4 | 5-16 | 17-24 | 25-32
            SLICES = [(132, 660), (660, 2244), (2244, 3300), (3300, 4356)]
            # stack slice sl becomes buildable once these casts are done;
            # JORD puts j=5,4 early so slice 2 (needed by next-step j=3)
            # fires 4 tiles before the step ends
            TRIGGER = {0: 0, 1: 1, 4: 2, 7: 3}  # cast j -> slice index
            JORD = [2, 3, 1, 5, 4, 0, 6, 7]

            def emit_slice(s, sl):
                """Stack-build DMAs (x8 -> stacks, dx baked) for step s+1."""
                nb = (s + 1) % 2
                ylo, yhi = SLICES[sl]
                for g in range(G):
                    sg = stk[g][nb]
                    for d in range(3):
                        eng = nc.sync if (g + d) % 2 == 0 else nc.gpsimd
                        eng.dma_start(
                            sg[24 * d : 24 * d + 24, ylo:yhi],
                            x8[32 * g : 32 * g + 24, ylo + d : yhi + d],
                        )

            def emit_fire(s):
                """Fire rows for step s+1, straight from DRAM."""
                nb = (s + 1) % 2
                for g in range(G):
                    s3 = stk[g][nb][:].rearrange("p (r w) -> p r w", w=PITCH)
                    nc.gpsimd.dma_start(
                        s3[72:73, 1:33, 0:128],
                        firein[32 * g + s + 1 : 32 * g + s + 2, :],
                    )

            def emit_edges(s):
                """Neighbor-stack halo rows for step s+1 (same partitions)."""
                nb = (s + 1) % 2
                for g in range(G):
                    sg = stk[g][nb]
                    sm = stk[(g - 1) % G][nb]
                    sp = stk[(g + 1) % G][nb]
                    nc.gpsimd.tensor_copy(sg[:73, 0:132], sm[:73, 4224:4356])
                    nc.gpsimd.tensor_copy(sg[:73, 4356:4488], sp[:73, 132:264])

            for s in range(STEPS):
                sb, nb = s % 2, (s + 1) % 2
                if s + 1 < STEPS:
                    emit_fire(s)
                stks = [
                    stk[g][sb][:].rearrange("p (r w) -> p r w", w=PITCH)
                    for g in range(G)
                ]

                def mm2_update(j, hss):
                    r0 = 4 * j + 1
                    dxt = dxpool.tile(
                        [128, TW], f32, tag="dx", name=f"dx_{s}_{j}"
                    )
                    for g in range(G):
                        nc.tensor.matmul(
                            dxt[32 * g : 32 * g + 32, :],
                            W2s[:],
                            hss[g][:],
                            start=True,
                            stop=True,
                            tile_position=(0, 32 * g),
                        )
                    # x += dx, in place (dx rows 24..31 of each band are 0)
                    dx3 = dxt[:].rearrange("p (a b) -> p a b", b=128)
                    nc.vector.tensor_tensor(
                        x16v[:, r0 : r0 + 4, 1:129],
                        dx3,
                        x16v[:, r0 : r0 + 4, 1:129],
                        Alu.add,
                    )
                    if s + 1 < STEPS:
                        # wrap columns then fp8 shadow of the updated rows
                        nc.scalar.activation(
                            x16v[:, r0 : r0 + 4, 0:1],
                            x16v[:, r0 : r0 + 4, 128:129],
                            Act.Copy,
                        )
                        nc.scalar.activation(
                            x16v[:, r0 : r0 + 4, 129:130],
                            x16v[:, r0 : r0 + 4, 1:2],
                            Act.Copy,
                        )
                        nc.scalar.activation(
                            x8v[:, r0 : r0 + 4, 0:130],
                            x16v[:, r0 : r0 + 4, 0:130],
                            Act.Copy,
                        )
                        if j in TRIGGER:
                            emit_slice(s, TRIGGER[j])
                    else:
                        nc.sync.dma_start(
                            xo3[:, 4 * j : 4 * j + 4, :],
                            x16v[:, r0 : r0 + 4, 1:129],
                        )

                prev = None
                for j in JORD:
                    r0 = 4 * j + 1
                    hts = [
                        hpool.tile([128, TW], f32, tag="h", name=f"h_{s}_{j}_{g}")
                        for g in range(G)
                    ]
                    # mm1: 3 dy-matmuls per group (fp8, K padded to 128);
                    # dy shift via the rhs AP, dx via the stack rows
                    for di, dy in enumerate((-1, 0, 1)):
                        lhsT = A[0:128, 128 * (dy + 1) : 128 * (dy + 2)]
                        for g in range(G):
                            rhs = stks[g][0:128, r0 + dy : r0 + dy + 4, 0:128]
                            nc.tensor.matmul(
                                hts[g][:, :],
                                lhsT,
                                rhs,
                                start=(di == 0),
                                stop=(di == 2),
                                tile_position=(0, 0),
                            )
                    hss = []
                    for g in range(G):
                        ht = hts[g]
                        hs = hsb.tile(
                            [128, TW], f16, tag="hsb", name=f"hs_{s}_{j}_{g}"
                        )
                        # h' = relu(h + 4(b1-56)); dy=0 fire row added 224*fire
                        nc.scalar.activation(
                            hs[:, :ASPL],
                            ht[:, :ASPL],
                            Act.Relu,
                            bias=b1c[:],
                        )
                        nc.vector.scalar_tensor_tensor(
                            hs[:, ASPL:],
                            ht[:, ASPL:],
                            b1c[:],
                            zeros[:],
                            Alu.add,
                            Alu.max,
                        )
                        hss.append(hs)
                    if prev is not None:
                        mm2_update(*prev)
                    prev = (j, hss)
                mm2_update(*prev)
                if s + 1 < STEPS:
                    emit_edges(s)

    nc.compile()
    return nc


def _get_module():
    if "nc" not in _CACHE:
        _CACHE["nc"] = _build_module()
    return _CACHE["nc"]


def _prep_weights(w1, w2, W1, b1, W2):
    A = np.zeros((9, HID, C), np.float32)
    for t in range(9):
        dy, dxx = t // 3 - 1, t % 3 - 1
        A[t] = (
            W1[:, 24:48] * w1[dy + 1, dxx + 1, 0][None, :]
            + W1[:, 48:72] * w2[dy + 1, dxx + 1, 0][None, :]
        )
    A[4] += W1[:, :24]
    apack = np.zeros((128, 384), np.float32)
    for d in range(3):
        for dyi in range(3):
            t = dyi * 3 + d
            apack[24 * d : 24 * d + 24, 128 * dyi : 128 * dyi + 128] = (
                WSCALE * A[t].T
            )
    apack[72, 128:256] = WSCALE * M_FIRE  # 224, exact in fp8e4m3
    w2pk = np.zeros((128, 32), np.float32)
    w2pk[:, NIC:C] = W2[NIC:C].T / WSCALE
    b1c = (WSCALE * (b1 - M_FIRE)).reshape(128, 1).astype(np.float32)
    return apack.astype(F8NP), w2pk.astype(np.float16), b1c


def _pack_x(ximg):
    """[128,128,24] image -> [128, FB] haloed channel-major."""
    xin = np.zeros((128, FB), np.float32)
    cols = (np.arange(-1, 129)) % WID
    for g in range(G):
        rows = (np.arange(-1, 33) + 32 * g) % H
        blk = ximg[rows][:, cols, :]  # [34, 130, 24]
        buf = np.zeros((24, GROWS, PITCH), np.float32)
        buf[:, :, :130] = np.transpose(blk, (2, 0, 1))
        xin[32 * g : 32 * g + 24] = buf.reshape(24, FB)
    return xin


def _unpack_x(xo):
    """[128, 4096] -> [128,128,24] image."""
    img = np.empty((H, WID, C), np.float32)
    for g in range(G):
        blk = xo[32 * g : 32 * g + 24].astype(np.float32).reshape(24, RG, WID)
        img[32 * g : 32 * g + 32] = np.transpose(blk, (1, 2, 0))
    return img


def _build_stack0(x8_0, fire0):
    """Host: step-0 stacks [128, G*FB] fp8 (x8_0 halos make edges free)."""
    stkin = np.zeros((128, G * FB), F8NP)
    for g in range(G):
        blk = stkin[:, g * FB : (g + 1) * FB]
        for d in range(3):
            blk[24 * d : 24 * d + 24, : FB - d] = x8_0[
                32 * g : 32 * g + 24, d:FB
            ]
        fr = fire0[32 * g : 32 * g + 32].reshape(RG * WID)
        f2 = blk[72].reshape(GROWS, PITCH)
        f2[1:33, 0:128] = fr.reshape(RG, WID)
    return stkin


def _make_in_maps(x, w1, w2, W1, b1, W2, rand_u):
    apack, w2pk, b1c = _prep_weights(w1, w2, W1, b1, W2)
    in_maps = []
    for b in range(B):
        fire = (rand_u[:, b, :, :, 0] < 0.5).astype(F8NP)  # [steps, H, W]
        firein = np.zeros((128, 4096), F8NP)
        for g in range(G):
            for s in range(STEPS):
                firein[32 * g + s] = fire[s, 32 * g : 32 * g + 32].reshape(4096)
        x16 = _pack_x(np.asarray(x[b], np.float32)).astype(np.float16)
        x8_0 = x16.astype(F8NP)
        in_maps.append(
            {
                "x16in": x16,
                "stk0in": _build_stack0(x8_0, fire[0]),
                "firein": firein,
                "apA": apack,
                "w2p": w2pk,
                "b1col": b1c,
            }
        )
    return in_maps


def kernel(x, w1, w2, W1, b1, W2, rand_u, steps, **kw):
    from concourse.bass_utils import run_bass_kernel_spmd

    assert int(steps) == STEPS
    x = np.asarray(x, np.float32)
    in_maps = _make_in_maps(
        x,
        np.asarray(w1, np.float32),
        np.asarray(w2, np.float32),
        np.asarray(W1, np.float32),
        np.asarray(b1, np.float32),
        np.asarray(W2, np.float32),
        np.asarray(rand_u, np.float32),
    )
    nc = _get_module()
    res = run_bass_kernel_spmd(nc, in_maps, core_ids=list(range(B)))
    _CACHE["last_results"] = res
    out = np.empty((B, H, WID, C), np.float32)
    for b in range(B):
        out[b] = _unpack_x(res.results[b]["xout"])
    return out


# revision 12
# speedup vs baseline: 1.1113x; 1.0276x over previous
"""Trainium2 Bass kernel for nn_BasicNCAModel (neural cellular automaton).

Sharding: data-parallel over batch B=8 across 8 NeuronCores (1 image/core).
kernel() takes full inputs, shards per image on the host, runs the SPMD Bass
module via run_bass_kernel_spmd (PJRT under axon), and reassembles.

Per-core design v2 (hardcoded for B=8, H=W=128, C=24, hidden=128, steps=8):
  - x lives ONLY as an fp16 master (x16, channel-major with halo: partition
    32g+c holds channel c of image rows [32g-1, 32g+32], 34 rows x 132 pitch).
    Updated in place; an fp8 shadow (x8) feeds the stack builds.
  - mm1 runs in fp8e4m3 (max 240): per-tap matrices A_t are scaled by 4,
    the fire coefficient is 224 = 4*56, bias is 4*(b1-56), and W2^T is
    scaled by 1/4, so dx is exact up to quantization. Masked pixels get
    relu(4(h+b1) - 224) = 0 (fire trick).
  - Per group a fp8 dx-stack (ping-ponged per step) holds rows 24d+c =
    x8 shifted by dx=d-1 (baked via contiguous DMA), row 72 = fire (DMA'd
    straight from DRAM per step), rows 73..127 = 0 (K padded to 128 for
    FWL full-rate matmuls). mm1 = 3 dy-matmuls per 512-pixel tile (dy via
    +-PITCH in the rhs AP); mm2 (fp16) is column-tiled so group g's dx
    lands at PSUM partitions 32g..32g+31 and x += dx is one in-place DVE
    add per tile.
  - Engine balance: Scalar = relu evac cols 0:256 + wrap-column fills +
    fp16->fp8 cast; Vector = STT evac cols 256:512 + the x update; GpSimd =
    stack-halo edge copies + fire DMAs + half the stack-slice DMAs; Sync =
    the other half + streamed output DMAs during the last step.
"""

import sys

if "/opt/trn_rl_repo" not in sys.path:
    sys.path.insert(0, "/opt/trn_rl_repo")

import ml_dtypes
import numpy as np

C = 24
NIC = 4
H = 128
WID = 128
HID = 128
STEPS = 8
B = 8
G = 4          # row groups
RG = 32        # image rows per group
PITCH = 132    # free-dim row pitch (130 used + 2 pad)
GROWS = 34     # rows incl halo
FB = GROWS * PITCH
TW = 512       # pixel tile = 4 image rows * 128 cols
JT = RG // 4   # tiles per group per step
WSCALE = 4.0   # weight scale so fire coeff fits fp8e4m3 (max 240)
M_FIRE = 56.0  # fire margin; coeff = WSCALE*M_FIRE = 224 (exact in fp8)
ASPL = 256     # evac split: scalar does [0:ASPL], vector [ASPL:512]

F8NP = ml_dtypes.float8_e4m3

_CACHE = {}


def _build_module():
    from concourse import bacc, mybir, tile

    f32 = mybir.dt.float32
    f16 = mybir.dt.float16
    f8 = mybir.dt.float8e4
    Alu = mybir.AluOpType
    Act = mybir.ActivationFunctionType

    nc = bacc.Bacc(
        "TRN2",
        target_bir_lowering=False,
        debug=False,
        enable_asserts=False,
        num_devices=8,
    )

    x16in = nc.dram_tensor("x16in", [128, FB], f16, kind="ExternalInput").ap()
    stk0in = nc.dram_tensor("stk0in", [128, G * FB], f8, kind="ExternalInput").ap()
    firein = nc.dram_tensor("firein", [128, 4096], f8, kind="ExternalInput").ap()
    apA = nc.dram_tensor("apA", [128, 384], f8, kind="ExternalInput").ap()
    w2p = nc.dram_tensor("w2p", [128, 32], f16, kind="ExternalInput").ap()
    b1col = nc.dram_tensor("b1col", [128, 1], f32, kind="ExternalInput").ap()
    xout = nc.dram_tensor("xout", [128, 4096], f16, kind="ExternalOutput").ap()

    with tile.TileContext(nc) as tc:
        import contextlib

        with contextlib.ExitStack() as ctx:
            sing = ctx.enter_context(tc.tile_pool(name="sing", bufs=1))
            hpool = ctx.enter_context(tc.tile_pool(name="h", bufs=6, space="PSUM"))
            dxpool = ctx.enter_context(tc.tile_pool(name="dx", bufs=2, space="PSUM"))
            hsb = ctx.enter_context(tc.tile_pool(name="hsb", bufs=8))

            x16 = sing.tile([128, FB], f16)
            x8 = sing.tile([128, FB], f8)
            A = sing.tile([128, 384], f8)
            W2s = sing.tile([128, 32], f16)
            zeros = sing.tile([128, TW - ASPL], f32)
            b1c = sing.tile([128, 1], f32)
            stk = [
                [sing.tile([128, FB], f8, name=f"stk_{g}_{p}") for p in range(2)]
                for g in range(G)
            ]

            # ---- preload: weights first (tiny), stacks sliced for early
            # start, x16 behind them (needed only at the first x update) ----
            nc.scalar.dma_start(A[:], apA[:])
            nc.scalar.dma_start(W2s[:], w2p[:])
            nc.scalar.dma_start(b1c[:], b1col[:])
            PSPLIT = 2640  # rows 0..19 first (covers the first ~4 tiles)
            for g in range(G):
                nc.sync.dma_start(
                    stk[g][0][:, 0:PSPLIT], stk0in[:, g * FB : g * FB + PSPLIT]
                )
            nc.scalar.dma_start(x16[:], x16in[:])
            for g in range(G):
                nc.gpsimd.dma_start(
                    stk[g][0][:, PSPLIT:FB],
                    stk0in[:, g * FB + PSPLIT : (g + 1) * FB],
                )
            for g in range(G):
                # odd-step stacks: zero rows 64+ (fire + K-padding; rows
                # 64..72 get overwritten by slice/fire DMAs each step)
                nc.vector.memset(stk[g][1][64:128, :], 0.0)
            nc.vector.memset(zeros[:], 0.0)

            x16v = x16[:].rearrange("p (r w) -> p r w", w=PITCH)
            x8v = x8[:].rearrange("p (r w) -> p r w", w=PITCH)
            xo3 = xout.rearrange("p (r w) -> p r w", w=128)
            # stack slice boundaries (flat y): rows 1-4 | 5-16 | 17-24 | 25-32
            SLICES = [(132, 660), (660, 2244), (2244, 3300), (3300, 4356)]
            # stack slice sl becomes buildable once these casts are done.
            # Step 0 runs j=1,2,3 first so slice 1 (which gates step 1's
            # first tiles) fires as early as the pipeline allows.
            TRIGGERS = {0: 0, 1: 1, 5: 2, 7: 3}  # cast j -> slice (steady)
            JORDS = [2, 3, 1, 4, 0, 5, 6, 7]
            TRIGGER0 = {0: 0, 3: 1, 4: 2, 7: 3}
            JORD0 = [1, 2, 3, 5, 4, 0, 6, 7]

            def emit_slice(s, sl):
                """Stack-build DMAs (x8 -> stacks, dx baked) for step s+1."""
                nb = (s + 1) % 2
                ylo, yhi = SLICES[sl]
                for g in range(G):
                    sg = stk[g][nb]
                    for d in range(3):
                        eng = nc.sync if (g + d) % 2 == 0 else nc.gpsimd
                        eng.dma_start(
                            sg[24 * d : 24 * d + 24, ylo:yhi],
                            x8[32 * g : 32 * g + 24, ylo + d : yhi + d],
                        )

            def emit_fire(s):
                """Fire rows for step s+1, straight from DRAM."""
                nb = (s + 1) % 2
                for g in range(G):
                    s3 = stk[g][nb][:].rearrange("p (r w) -> p r w", w=PITCH)
                    nc.gpsimd.dma_start(
                        s3[72:73, 1:33, 0:128],
                        firein[32 * g + s + 1 : 32 * g + s + 2, :],
                    )

            def emit_edges(s):
                """Neighbor-stack halo rows for step s+1 (same partitions)."""
                nb = (s + 1) % 2
                for g in range(G):
                    sg = stk[g][nb]
                    sm = stk[(g - 1) % G][nb]
                    sp = stk[(g + 1) % G][nb]
                    nc.gpsimd.tensor_copy(sg[:73, 0:132], sm[:73, 4224:4356])
                    nc.gpsimd.tensor_copy(sg[:73, 4356:4488], sp[:73, 132:264])

            for s in range(STEPS):
                sb, nb = s % 2, (s + 1) % 2
                JORD = JORD0 if s == 0 else JORDS
                TRIGGER = TRIGGER0 if s == 0 else TRIGGERS
                stks = [
                    stk[g][sb][:].rearrange("p (r w) -> p r w", w=PITCH)
                    for g in range(G)
                ]

                def mm2_update(j, hss):
                    r0 = 4 * j + 1
                    dxt = dxpool.tile(
                        [128, TW], f32, tag="dx", name=f"dx_{s}_{j}"
                    )
                    for g in range(G):
                        nc.tensor.matmul(
                            dxt[32 * g : 32 * g + 32, :],
                            W2s[:],
                            hss[g][:],
                            start=True,
                            stop=True,
                            tile_position=(0, 32 * g),
                        )
                    # x += dx, in place (dx rows 24..31 of each band are 0)
                    dx3 = dxt[:].rearrange("p (a b) -> p a b", b=128)
                    nc.vector.tensor_tensor(
                        x16v[:, r0 : r0 + 4, 1:129],
                        dx3,
                        x16v[:, r0 : r0 + 4, 1:129],
                        Alu.add,
                    )
                    if s + 1 < STEPS:
                        # wrap columns then fp8 shadow of the updated rows
                        nc.scalar.activation(
                            x16v[:, r0 : r0 + 4, 0:1],
                            x16v[:, r0 : r0 + 4, 128:129],
                            Act.Copy,
                        )
                        nc.scalar.activation(
                            x16v[:, r0 : r0 + 4, 129:130],
                            x16v[:, r0 : r0 + 4, 1:2],
                            Act.Copy,
                        )
                        nc.scalar.activation(
                            x8v[:, r0 : r0 + 4, 0:130],
                            x16v[:, r0 : r0 + 4, 0:130],
                            Act.Copy,
                        )
                        if j in TRIGGER:
                            emit_slice(s, TRIGGER[j])
                    else:
                        nc.sync.dma_start(
                            xo3[:, 4 * j : 4 * j + 4, :],
                            x16v[:, r0 : r0 + 4, 1:129],
                        )

                prev = None
                for ji, j in enumerate(JORD):
                    if ji == 3 and s + 1 < STEPS:
                        emit_fire(s)  # mid-step: off the slice DMAs' way
                    r0 = 4 * j + 1
                    hts = [
                        hpool.tile([128, TW], f32, tag="h", name=f"h_{s}_{j}_{g}")
                        for g in range(G)
                    ]
                    # mm1: 3 dy-matmuls per group (fp8, K padded to 128);
                    # dy shift via the rhs AP, dx via the stack rows
                    for di, dy in enumerate((-1, 0, 1)):
                        lhsT = A[0:128, 128 * (dy + 1) : 128 * (dy + 2)]
                        for g in range(G):
                            rhs = stks[g][0:128, r0 + dy : r0 + dy + 4, 0:128]
                            nc.tensor.matmul(
                                hts[g][:, :],
                                lhsT,
                                rhs,
                                start=(di == 0),
                                stop=(di == 2),
                                tile_position=(0, 0),
                            )
                    hss = []
                    for g in range(G):
                        ht = hts[g]
                        hs = hsb.tile(
                            [128, TW], f16, tag="hsb", name=f"hs_{s}_{j}_{g}"
                        )
                        # h' = relu(h + 4(b1-56)); dy=0 fire row added 224*fire
                        nc.scalar.activation(
                            hs[:, :ASPL],
                            ht[:, :ASPL],
                            Act.Relu,
                            bias=b1c[:],
                        )
                        nc.vector.scalar_tensor_tensor(
                            hs[:, ASPL:],
                            ht[:, ASPL:],
                            b1c[:],
                            zeros[:],
                            Alu.add,
                            Alu.max,
                        )
                        hss.append(hs)
                    if prev is not None:
                        mm2_update(*prev)
                    prev = (j, hss)
                mm2_update(*prev)
                if s + 1 < STEPS:
                    emit_edges(s)

    nc.compile()
    return nc


def _get_module():
    if "nc" not in _CACHE:
        _CACHE["nc"] = _build_module()
    return _CACHE["nc"]


def _prep_weights(w1, w2, W1, b1, W2):
    A = np.zeros((9, HID, C), np.float32)
    for t in range(9):
        dy, dxx = t // 3 - 1, t % 3 - 1
        A[t] = (
            W1[:, 24:48] * w1[dy + 1, dxx + 1, 0][None, :]
            + W1[:, 48:72] * w2[dy + 1, dxx + 1, 0][None, :]
        )
    A[4] += W1[:, :24]
    apack = np.zeros((128, 384), np.float32)
    for d in range(3):
        for dyi in range(3):
            t = dyi * 3 + d
            apack[24 * d : 24 * d + 24, 128 * dyi : 128 * dyi + 128] = (
                WSCALE * A[t].T
            )
    apack[72, 128:256] = WSCALE * M_FIRE  # 224, exact in fp8e4m3
    w2pk = np.zeros((128, 32), np.float32)
    w2pk[:, NIC:C] = W2[NIC:C].T / WSCALE
    b1c = (WSCALE * (b1 - M_FIRE)).reshape(128, 1).astype(np.float32)
    return apack.astype(F8NP), w2pk.astype(np.float16), b1c


def _pack_x(ximg):
    """[128,128,24] image -> [128, FB] haloed channel-major."""
    xin = np.zeros((128, FB), np.float32)
    cols = (np.arange(-1, 129)) % WID
    for g in range(G):
        rows = (np.arange(-1, 33) + 32 * g) % H
        blk = ximg[rows][:, cols, :]  # [34, 130, 24]
        buf = np.zeros((24, GROWS, PITCH), np.float32)
        buf[:, :, :130] = np.transpose(blk, (2, 0, 1))
        xin[32 * g : 32 * g + 24] = buf.reshape(24, FB)
    return xin


def _unpack_x(xo):
    """[128, 4096] -> [128,128,24] image."""
    img = np.empty((H, WID, C), np.float32)
    for g in range(G):
        blk = xo[32 * g : 32 * g + 24].astype(np.float32).reshape(24, RG, WID)
        img[32 * g : 32 * g + 32] = np.transpose(blk, (1, 2, 0))
    return img


def _build_stack0(x8_0, fire0):
    """Host: step-0 stacks [128, G*FB] fp8 (x8_0 halos make edges free)."""
    stkin = np.zeros((128, G * FB), F8NP)
    for g in range(G):
        blk = stkin[:, g * FB : (g + 1) * FB]
        for d in range(3):
            blk[24 * d : 24 * d + 24, : FB - d] = x8_0[
                32 * g : 32 * g + 24, d:FB
            ]
        fr = fire0[32 * g : 32 * g + 32].reshape(RG * WID)
        f2 = blk[72].reshape(GROWS, PITCH)
        f2[1:33, 0:128] = fr.reshape(RG, WID)
    return stkin


def _make_in_maps(x, w1, w2, W1, b1, W2, rand_u):
    apack, w2pk, b1c = _prep_weights(w1, w2, W1, b1, W2)
    in_maps = []
    for b in range(B):
        fire = (rand_u[:, b, :, :, 0] < 0.5).astype(F8NP)  # [steps, H, W]
        firein = np.zeros((128, 4096), F8NP)
        for g in range(G):
            for s in range(STEPS):
                firein[32 * g + s] = fire[s, 32 * g : 32 * g + 32].reshape(4096)
        x16 = _pack_x(np.asarray(x[b], np.float32)).astype(np.float16)
        x8_0 = x16.astype(F8NP)
        in_maps.append(
            {
                "x16in": x16,
                "stk0in": _build_stack0(x8_0, fire[0]),
                "firein": firein,
                "apA": apack,
                "w2p": w2pk,
                "b1col": b1c,
            }
        )
    return in_maps


def kernel(x, w1, w2, W1, b1, W2, rand_u, steps, **kw):
    from concourse.bass_utils import run_bass_kernel_spmd

    assert int(steps) == STEPS
    x = np.asarray(x, np.float32)
    in_maps = _make_in_maps(
        x,
        np.asarray(w1, np.float32),
        np.asarray(w2, np.float32),
        np.asarray(W1, np.float32),
        np.asarray(b1, np.float32),
        np.asarray(W2, np.float32),
        np.asarray(rand_u, np.float32),
    )
    nc = _get_module()
    res = run_bass_kernel_spmd(nc, in_maps, core_ids=list(range(B)))
    _CACHE["last_results"] = res
    out = np.empty((B, H, WID, C), np.float32)
    for b in range(B):
        out[b] = _unpack_x(res.results[b]["xout"])
    return out
